# revision 7
# baseline (speedup 1.0000x reference)
"""CrossViewAttention Trainium2 kernel.

Shards the B*V=16 (batch, view) attention instances across 8 NeuronCores
(2 per core, data-parallel; weights replicated). The circular neighbor
gather (views v-1, v+1) is resolved on the host when slicing per-core
inputs, so no device collectives are needed.

Per core, for each of its 2 pairs:
  Q^T = wq.T @ x^T           (fp32r matmuls, d-contraction)
  K^T = wk.T @ x_kv^T        V = x_kv @ wv  (natural layout, +ones col)
  scores^T[t,s] = K^T.T @ Q^T   per head (GQA: head h uses kv head h//4)
  E = exp(scale*scores^T)    (no max subtraction; scores are O(1))
  [O^T; l] = V_aug.T @ E     (ones column folds the softmax denominator)
  O^T *= broadcast(1/l)      (K=1 ones matmul broadcasts 1/l over hd)
  y = O @ wo
"""
import numpy as np

B, V, S, D = 2, 8, 256, 2048
NH, NKV, KVR = 32, 8, 2
HD = D // NH  # 64
G = NH // NKV  # 4
N_CORES = 8
P = 2  # pairs per core
SCALE = 1.0 / np.sqrt(HD)

_CACHE = {}


def _to_f32r(a: np.ndarray) -> np.ndarray:
    """Round fp32 to the fp32r format (e8m11, RNE): low 12 bits zeroed."""
    u = np.ascontiguousarray(a, dtype=np.float32).view(np.uint32)
    u = (u + 0x7FF + ((u >> 12) & 1)) & 0xFFFFF000
    return u.view(np.float32)


def _build():
    import concourse.bass as bass
    import concourse.tile as tile
    import concourse.mybir as mybir
    from concourse import bacc
    from contextlib import ExitStack

    F32 = mybir.dt.float32
    F32R = mybir.dt.float32r

    nc = bacc.Bacc("TRN2", target_bir_lowering=False, debug=False,
                   num_devices=N_CORES)
    xqT = nc.dram_tensor("xqT", [D, P * S], F32R, kind="ExternalInput").ap()
    xkvT = nc.dram_tensor("xkvT", [D, P * 512], F32R, kind="ExternalInput").ap()
    wq = nc.dram_tensor("wq", [D, D], F32R, kind="ExternalInput").ap()
    wkv = nc.dram_tensor("wkv", [D, 1024], F32R, kind="ExternalInput").ap()
    wo = nc.dram_tensor("wo", [D, D], F32R, kind="ExternalInput").ap()
    ones1 = nc.dram_tensor("ones1", [1, HD], F32R, kind="ExternalInput").ap()
    vones = nc.dram_tensor("vones", [128, 8], F32R, kind="ExternalInput").ap()
    y = nc.dram_tensor("y", [P * S, D], F32, kind="ExternalOutput").ap()

    with tile.TileContext(nc) as tc, ExitStack() as top:
        misc = top.enter_context(tc.tile_pool(name="misc", bufs=2))
        ktp = top.enter_context(tc.tile_pool(name="ktp", bufs=1))
        vp = top.enter_context(tc.tile_pool(name="vp", bufs=1))

        on_sb = misc.tile([1, HD], F32R, tag="ones")
        nc.gpsimd.dma_start(on_sb[:], ones1[:])
        vo_sb = misc.tile([128, 8], F32R, tag="vones")
        nc.gpsimd.dma_start(vo_sb[:], vones[:])

        KT = [ktp.tile([64, 2048], F32R, tag=f"kt{i}", name=f"kt{i}") for i in range(4)]
        VA = [[vp.tile([128, 8 * 65], F32R, tag=f"va{p}_{t}", name=f"va{p}_{t}") for t in range(4)]
              for p in range(P)]

        # ---------- Phase A1/A2: K^T, V (uses xkvT; wv resident) ----------
        with ExitStack() as ph:
            xkp = ph.enter_context(tc.tile_pool(name="xkp", bufs=1))
            wvp = ph.enter_context(tc.tile_pool(name="wvp", bufs=3))
            wst = ph.enter_context(tc.tile_pool(name="wst", bufs=3))
            psA = ph.enter_context(tc.tile_pool(name="psA", bufs=4, space="PSUM"))

            xkv = []
            for k in range(16):
                t = xkp.tile([128, 1024], F32R, tag=f"xkv{k}", name=f"xkv{k}")
                nc.gpsimd.dma_start(t[:], xkvT[k * 128:(k + 1) * 128, :])
                xkv.append(t)

            # A1: K^T[f, t] per f-tile; psum per pair
            for fk in range(4):
                ps0 = psA.tile([128, 512], F32, tag="pa")
                ps1 = psA.tile([128, 512], F32, tag="pa")
                for k in range(16):
                    wt = wst.tile([128, 128], F32R, tag="wk")
                    nc.gpsimd.dma_start(
                        wt[:], wkv[k * 128:(k + 1) * 128, fk * 128:(fk + 1) * 128])
                    nc.tensor.matmul(ps0[:], wt[:], xkv[k][:, 0:512],
                                     start=(k == 0), stop=(k == 15))
                    nc.tensor.matmul(ps1[:], wt[:], xkv[k][:, 512:1024],
                                     start=(k == 0), stop=(k == 15))
                nc.vector.tensor_copy(KT[fk][0:64, 0:512], ps0[0:64, :])
                nc.vector.tensor_copy(KT[fk][0:64, 1024:1536], ps0[64:128, :])
                nc.vector.tensor_copy(KT[fk][0:64, 512:1024], ps1[0:64, :])
                nc.vector.tensor_copy(KT[fk][0:64, 1536:2048], ps1[64:128, :])

            # A2: V natural [t, f] + ones; k-outer per pair, wv streamed 2x
            for p in range(P):
                vps = [psA.tile([128, 512], F32, tag="pa", name=f"pvv{p}_{i}")
                       for i in range(4)]
                for k in range(16):
                    wvt = wvp.tile([128, 512], F32R, tag="wv")
                    nc.gpsimd.dma_start(wvt[:], wkv[k * 128:(k + 1) * 128, 512:1024])
                    for tt in range(4):
                        nc.tensor.matmul(
                            vps[tt][:],
                            xkv[k][:, p * 512 + tt * 128:p * 512 + (tt + 1) * 128],
                            wvt[:],
                            start=(k == 0), stop=(k == 15))
                for tt in range(4):
                    ps = vps[tt]
                    va = VA[p][tt]
                    dst = va[:].rearrange("q (h c) -> q h c", c=65)[:, :, 0:64]
                    src = ps[:].rearrange("q (h c) -> q h c", c=64)
                    nc.vector.tensor_copy(dst, src)
                    od = va[:].rearrange("q (h c) -> q h c", c=65)[:, :, 64:65]
                    nc.vector.tensor_copy(od, vo_sb[:].unsqueeze(2))

        # ---------- Phase A3: Q^T (uses xqT) ----------
        qtp = top.enter_context(tc.tile_pool(name="qtp", bufs=1))
        QT = [qtp.tile([64, 1024], F32R, tag=f"qt{j}", name=f"qt{j}")
              for j in range(16)]
        otp = top.enter_context(tc.tile_pool(name="otp", bufs=1))
        OT = [[otp.tile([128, 256], F32R, tag=f"ot{p}_{i}", name=f"ot{p}_{i}") for i in range(16)]
              for p in range(P)]

        with ExitStack() as ph:
            xqp = ph.enter_context(tc.tile_pool(name="xqp", bufs=1))
            wst = ph.enter_context(tc.tile_pool(name="wst2", bufs=3))
            psA = ph.enter_context(tc.tile_pool(name="psA2", bufs=4, space="PSUM"))

            xq = []
            for k in range(16):
                t = xqp.tile([128, 512], F32R, tag=f"xq{k}", name=f"xq{k}")
                nc.gpsimd.dma_start(t[:], xqT[k * 128:(k + 1) * 128, :])
                xq.append(t)

            for fq in range(16):
                ps = psA.tile([128, 512], F32, tag="pa")
                for k in range(16):
                    wt = wst.tile([128, 128], F32R, tag="wq")
                    nc.gpsimd.dma_start(
                        wt[:], wq[k * 128:(k + 1) * 128, fq * 128:(fq + 1) * 128])
                    nc.tensor.matmul(ps[:], wt[:], xq[k][:],
                                     start=(k == 0), stop=(k == 15))
                # rows 0:64 = head 2fq, rows 64:128 = head 2fq+1
                for half, h in ((0, 2 * fq), (1, 2 * fq + 1)):
                    n, g = h // 4, h % 4
                    j = (n // 2) * 4 + g
                    blk = (n % 2) * 512
                    nc.vector.tensor_copy(
                        QT[j][0:64, blk:blk + 512],
                        ps[half * 64:(half + 1) * 64, :])

        # ---------- Phase B: attention ----------
        with ExitStack() as ph:
            ep = ph.enter_context(tc.tile_pool(name="ep", bufs=8))
            lp = ph.enter_context(tc.tile_pool(name="lp", bufs=4))
            rp = ph.enter_context(tc.tile_pool(name="rp", bufs=4))
            qkps = ph.enter_context(tc.tile_pool(name="qkps", bufs=5, space="PSUM"))
            pvps = ph.enter_context(tc.tile_pool(name="pvps", bufs=2, space="PSUM"))
            rps = ph.enter_context(tc.tile_pool(name="rps", bufs=1, space="PSUM"))

            for j in range(16):
                a = j // 4
                g = j % 4
                jk = j // 4
                hA = 8 * a + g
                hB = 8 * a + 4 + g
                nA = 2 * a
                nB = 2 * a + 1
                for p in range(P):
                    # QK^T: per t-tile, heads A (rows 0:64) and B (64:128)
                    es = []
                    for tt in range(4):
                        qk = qkps.tile([128, 512], F32, tag="qk")
                        cA = p * 512 + tt * 128
                        nc.tensor.matmul(
                            qk[:, 0:256],
                            KT[jk][0:64, cA:cA + 128],
                            QT[j][0:64, p * 256:(p + 1) * 256],
                            start=True, stop=True)
                        nc.tensor.matmul(
                            qk[:, 256:512],
                            KT[jk][0:64, 1024 + cA:1024 + cA + 128],
                            QT[j][0:64, 512 + p * 256:512 + (p + 1) * 256],
                            start=True, stop=True)
                        e = ep.tile([128, 512], F32R, tag="e")
                        nc.scalar.activation(
                            e[:], qk[:], mybir.ActivationFunctionType.Exp,
                            scale=float(SCALE))
                        es.append(e)

                    # PV with ones-fold: [65, 512] = [O^T_A | O^T_B ; l]
                    pv = pvps.tile([65, 512], F32, tag="pv")
                    for nh, c0 in ((nA, 0), (nB, 256)):
                        for tt in range(4):
                            nc.tensor.matmul(
                                pv[:, c0:c0 + 256],
                                VA[p][tt][:, nh * 65:(nh + 1) * 65],
                                es[tt][:, c0:c0 + 256],
                                start=(tt == 0), stop=(tt == 3))

                    # softmax denominators -> broadcast reciprocal
                    l2 = lp.tile([1, 512], F32, tag="l2")
                    nc.vector.tensor_copy(l2[0:1, 0:256], pv[64:65, 0:256])
                    nc.vector.tensor_copy(l2[0:1, 256:512], pv[64:65, 256:512])
                    r2 = lp.tile([1, 512], F32R, tag="r2")
                    with nc.allow_low_precision(reason="fp32r matmul input"):
                        nc.vector.reciprocal(r2[:], l2[:])
                    rb = rps.tile([64, 512], F32, tag="rb")
                    nc.tensor.matmul(rb[:, 0:256], on_sb[:], r2[0:1, 0:256],
                                     start=True, stop=True)
                    nc.tensor.matmul(rb[:, 256:512], on_sb[:], r2[0:1, 256:512],
                                     start=True, stop=True)
                    rsb = rp.tile([64, 512], F32, tag="rsb")
                    nc.vector.tensor_copy(rsb[:], rb[:])

                    # normalize + scatter to O^T tiles
                    for h, c0 in ((hA, 0), (hB, 256)):
                        ot = OT[p][h // 2]
                        ob = (h % 2) * 64
                        nc.vector.tensor_tensor(
                            ot[ob:ob + 64, :],
                            pv[0:64, c0:c0 + 256],
                            rsb[0:64, c0:c0 + 256],
                            mybir.AluOpType.mult)

        # ---------- Phase C: output projection ----------
        with ExitStack() as ph:
            wop = ph.enter_context(tc.tile_pool(name="wop", bufs=3))
            yst = ph.enter_context(tc.tile_pool(name="yst", bufs=4))
            psC = ph.enter_context(tc.tile_pool(name="psC", bufs=8, space="PSUM"))

            for nn in range(4):
                acc = [[psC.tile([128, 512], F32, tag="pc", name=f"pc{nn}_{m}") for m in range(2)]
                       for p in range(P)]
                for k in range(16):
                    wt = wop.tile([128, 512], F32R, tag="wo")
                    nc.gpsimd.dma_start(
                        wt[:], wo[k * 128:(k + 1) * 128, nn * 512:(nn + 1) * 512])
                    for p in range(P):
                        for m in range(2):
                            nc.tensor.matmul(
                                acc[p][m][:],
                                OT[p][k][:, m * 128:(m + 1) * 128],
                                wt[:],
                                start=(k == 0), stop=(k == 15))
                for p in range(P):
                    for m in range(2):
                        yt = yst.tile([128, 512], F32, tag="yt")
                        nc.vector.tensor_copy(yt[:], acc[p][m][:])
                        r0 = p * 256 + m * 128
                        nc.gpsimd.dma_start(
                            y[r0:r0 + 128, nn * 512:(nn + 1) * 512], yt[:])

    nc.compile()
    return nc


def _get_nc():
    if "nc" not in _CACHE:
        _CACHE["nc"] = _build()
    return _CACHE["nc"]


def kernel(x, wq, wkv, wo):
    from concourse.bass_utils import run_bass_kernel_spmd

    nc = _get_nc()
    x = np.asarray(x, dtype=np.float32)
    wq_r = _to_f32r(wq)
    wkv_r = _to_f32r(wkv)
    wo_r = _to_f32r(wo)
    ones1 = np.ones((1, HD), np.float32)
    vones = np.ones((128, 8), np.float32)

    in_maps = []
    for c in range(N_CORES):
        xq_cols = []
        xkv_cols = []
        for p in range(P):
            pg = 2 * c + p
            b, v = pg // V, pg % V
            xq_cols.append(np.ascontiguousarray(x[b, v].T))
            xkv_cols.append(np.ascontiguousarray(
                np.concatenate([x[b, (v - 1) % V], x[b, (v + 1) % V]], axis=0).T))
        in_maps.append({
            "xqT": _to_f32r(np.concatenate(xq_cols, axis=1)),
            "xkvT": _to_f32r(np.concatenate(xkv_cols, axis=1)),
            "wq": wq_r, "wkv": wkv_r, "wo": wo_r, "ones1": ones1, "vones": vones,
        })

    res = run_bass_kernel_spmd(nc, in_maps, list(range(N_CORES)),
                               trace=False)
    out = np.empty((B, V, S, D), np.float32)
    for c in range(N_CORES):
        yc = res.results[c]["y"]
        for p in range(P):
            pg = 2 * c + p
            b, v = pg // V, pg % V
            out[b, v] = yc[p * S:(p + 1) * S]
    return out


# revision 8
# speedup vs baseline: 1.5237x; 1.5237x over previous
"""CrossViewAttention Trainium2 kernel.

Shards the B*V=16 (batch, view) attention instances across 8 NeuronCores
(2 per core, data-parallel; weights replicated). The circular neighbor
gather (views v-1, v+1) is resolved on the host when slicing per-core
inputs, so no device collectives are needed.

Per core, for each of its 2 pairs:
  Q^T = wq.T @ x^T           (fp32r matmuls, d-contraction)
  K^T = wk.T @ x_kv^T        V = x_kv @ wv  (natural layout, +ones col)
  scores^T[t,s] = K^T.T @ Q^T   per head (GQA: head h uses kv head h//4)
  E = exp(scale*scores^T)    (no max subtraction; scores are O(1))
  [O^T; l] = V_aug.T @ E     (ones column folds the softmax denominator)
  O^T *= broadcast(1/l)      (K=1 ones matmul broadcasts 1/l over hd)
  y = O @ wo
"""
import numpy as np

B, V, S, D = 2, 8, 256, 2048
NH, NKV, KVR = 32, 8, 2
HD = D // NH  # 64
G = NH // NKV  # 4
N_CORES = 8
P = 2  # pairs per core
SCALE = 1.0 / np.sqrt(HD)

_CACHE = {}


def _to_f32r(a: np.ndarray) -> np.ndarray:
    """Round fp32 to the fp32r format (e8m11, RNE): low 12 bits zeroed."""
    u = np.ascontiguousarray(a, dtype=np.float32).view(np.uint32)
    u = (u + 0x7FF + ((u >> 12) & 1)) & 0xFFFFF000
    return u.view(np.float32)


def _build():
    import concourse.bass as bass
    import concourse.tile as tile
    import concourse.mybir as mybir
    from concourse import bacc
    from contextlib import ExitStack

    F32 = mybir.dt.float32
    F32R = mybir.dt.float32r

    nc = bacc.Bacc("TRN2", target_bir_lowering=False, debug=False,
                   num_devices=N_CORES)
    xqT = nc.dram_tensor("xqT", [D, P * S], F32R, kind="ExternalInput").ap()
    xkvT = nc.dram_tensor("xkvT", [D, P * 512], F32R, kind="ExternalInput").ap()
    wq = nc.dram_tensor("wq", [D, D], F32R, kind="ExternalInput").ap()
    wkv = nc.dram_tensor("wkv", [D, 1024], F32R, kind="ExternalInput").ap()
    wo = nc.dram_tensor("wo", [D, D], F32R, kind="ExternalInput").ap()
    ones1 = nc.dram_tensor("ones1", [1, HD], F32R, kind="ExternalInput").ap()
    vones = nc.dram_tensor("vones", [128, 8], F32R, kind="ExternalInput").ap()
    y = nc.dram_tensor("y", [P * S, D], F32, kind="ExternalOutput").ap()

    with tile.TileContext(nc) as tc, ExitStack() as top:
        misc = top.enter_context(tc.tile_pool(name="misc", bufs=2))
        ktp = top.enter_context(tc.tile_pool(name="ktp", bufs=1))
        vp = top.enter_context(tc.tile_pool(name="vp", bufs=1))

        on_sb = misc.tile([1, HD], F32R, tag="ones")
        nc.gpsimd.dma_start(on_sb[:], ones1[:])
        vo_sb = misc.tile([128, 8], F32R, tag="vones")
        nc.gpsimd.dma_start(vo_sb[:], vones[:])

        KT = [ktp.tile([64, 2048], F32R, tag=f"kt{i}", name=f"kt{i}") for i in range(4)]
        VA = [[vp.tile([128, 8 * 65], F32R, tag=f"va{p}_{t}", name=f"va{p}_{t}") for t in range(4)]
              for p in range(P)]

        # ---------- Phase A1/A2: K^T, V (uses xkvT; wv resident) ----------
        with ExitStack() as ph:
            xkp = ph.enter_context(tc.tile_pool(name="xkp", bufs=1))
            wvp = ph.enter_context(tc.tile_pool(name="wvp", bufs=3))
            wst = ph.enter_context(tc.tile_pool(name="wst", bufs=3))
            psA = ph.enter_context(tc.tile_pool(name="psA", bufs=8, space="PSUM"))

            xkv = []
            for k in range(16):
                t = xkp.tile([128, 1024], F32R, tag=f"xkv{k}", name=f"xkv{k}")
                nc.sync.dma_start(t[:], xkvT[k * 128:(k + 1) * 128, :])
                xkv.append(t)

            # A1: K^T[f, t]; k outer, batched wk loads, 8 accumulators
            kps = [psA.tile([128, 512], F32, tag="pa", name=f"kps{i}")
                   for i in range(8)]
            for k in range(16):
                wt = wst.tile([128, 512], F32R, tag="wk")
                nc.sync.dma_start(wt[:], wkv[k * 128:(k + 1) * 128, 0:512])
                for fk in range(4):
                    nc.tensor.matmul(kps[fk * 2][:],
                                     wt[:, fk * 128:(fk + 1) * 128],
                                     xkv[k][:, 0:512],
                                     start=(k == 0), stop=(k == 15))
                    nc.tensor.matmul(kps[fk * 2 + 1][:],
                                     wt[:, fk * 128:(fk + 1) * 128],
                                     xkv[k][:, 512:1024],
                                     start=(k == 0), stop=(k == 15))
            for fk in range(4):
                ps0, ps1 = kps[fk * 2], kps[fk * 2 + 1]
                nc.vector.tensor_copy(KT[fk][0:64, 0:512], ps0[0:64, :])
                nc.vector.tensor_copy(KT[fk][0:64, 1024:1536], ps0[64:128, :])
                nc.vector.tensor_copy(KT[fk][0:64, 512:1024], ps1[0:64, :])
                nc.vector.tensor_copy(KT[fk][0:64, 1536:2048], ps1[64:128, :])

            # A2: V natural [t, f] + ones; k-outer per pair, wv streamed 2x
            for p in range(P):
                vps = [psA.tile([128, 512], F32, tag="pa", name=f"pvv{p}_{i}")
                       for i in range(4)]
                for k in range(16):
                    wvt = wvp.tile([128, 512], F32R, tag="wv")
                    nc.sync.dma_start(wvt[:], wkv[k * 128:(k + 1) * 128, 512:1024])
                    for tt in range(4):
                        nc.tensor.matmul(
                            vps[tt][:],
                            xkv[k][:, p * 512 + tt * 128:p * 512 + (tt + 1) * 128],
                            wvt[:],
                            start=(k == 0), stop=(k == 15))
                for tt in range(4):
                    ps = vps[tt]
                    va = VA[p][tt]
                    dst = va[:].rearrange("q (h c) -> q h c", c=65)[:, :, 0:64]
                    src = ps[:].rearrange("q (h c) -> q h c", c=64)
                    nc.vector.tensor_copy(dst, src)
                    od = va[:].rearrange("q (h c) -> q h c", c=65)[:, :, 64:65]
                    nc.vector.tensor_copy(od, vo_sb[:].unsqueeze(2))

        # ---------- Phase A3: Q^T (uses xqT) ----------
        qtp = top.enter_context(tc.tile_pool(name="qtp", bufs=1))
        QT = [qtp.tile([64, 1024], F32R, tag=f"qt{j}", name=f"qt{j}")
              for j in range(16)]
        otp = top.enter_context(tc.tile_pool(name="otp", bufs=1))
        OT = [[otp.tile([128, 256], F32R, tag=f"ot{p}_{i}", name=f"ot{p}_{i}") for i in range(16)]
              for p in range(P)]

        with ExitStack() as ph:
            xqp = ph.enter_context(tc.tile_pool(name="xqp", bufs=1))
            wst = ph.enter_context(tc.tile_pool(name="wst2", bufs=3))
            psA = ph.enter_context(tc.tile_pool(name="psA2", bufs=8, space="PSUM"))

            xq = []
            for k in range(16):
                t = xqp.tile([128, 512], F32R, tag=f"xq{k}", name=f"xq{k}")
                nc.sync.dma_start(t[:], xqT[k * 128:(k + 1) * 128, :])
                xq.append(t)

            for fg in range(4):
                qps = [psA.tile([128, 512], F32, tag="pa", name=f"qps{fg}_{i}")
                       for i in range(4)]
                for k in range(16):
                    wt = wst.tile([128, 512], F32R, tag="wq")
                    nc.sync.dma_start(
                        wt[:], wq[k * 128:(k + 1) * 128, fg * 512:(fg + 1) * 512])
                    for fi in range(4):
                        nc.tensor.matmul(qps[fi][:],
                                         wt[:, fi * 128:(fi + 1) * 128],
                                         xq[k][:],
                                         start=(k == 0), stop=(k == 15))
                for fi in range(4):
                    fq = fg * 4 + fi
                    ps = qps[fi]
                    for half, h in ((0, 2 * fq), (1, 2 * fq + 1)):
                        n, g = h // 4, h % 4
                        j = (n // 2) * 4 + g
                        blk = (n % 2) * 512
                        nc.vector.tensor_copy(
                            QT[j][0:64, blk:blk + 512],
                            ps[half * 64:(half + 1) * 64, :])

        # ---------- Phase B: attention ----------
        with ExitStack() as ph:
            ep = ph.enter_context(tc.tile_pool(name="ep", bufs=8))
            lp = ph.enter_context(tc.tile_pool(name="lp", bufs=4))
            rp = ph.enter_context(tc.tile_pool(name="rp", bufs=4))
            qkps = ph.enter_context(tc.tile_pool(name="qkps", bufs=5, space="PSUM"))
            pvps = ph.enter_context(tc.tile_pool(name="pvps", bufs=2, space="PSUM"))
            rps = ph.enter_context(tc.tile_pool(name="rps", bufs=1, space="PSUM"))

            for j in range(16):
                a = j // 4
                g = j % 4
                jk = j // 4
                hA = 8 * a + g
                hB = 8 * a + 4 + g
                nA = 2 * a
                nB = 2 * a + 1
                for p in range(P):
                    # QK^T: per t-tile, heads A (rows 0:64) and B (64:128)
                    es = []
                    for tt in range(4):
                        qk = qkps.tile([128, 512], F32, tag="qk")
                        cA = p * 512 + tt * 128
                        nc.tensor.matmul(
                            qk[:, 0:256],
                            KT[jk][0:64, cA:cA + 128],
                            QT[j][0:64, p * 256:(p + 1) * 256],
                            start=True, stop=True)
                        nc.tensor.matmul(
                            qk[:, 256:512],
                            KT[jk][0:64, 1024 + cA:1024 + cA + 128],
                            QT[j][0:64, 512 + p * 256:512 + (p + 1) * 256],
                            start=True, stop=True)
                        e = ep.tile([128, 512], F32R, tag="e")
                        nc.scalar.activation(
                            e[:], qk[:], mybir.ActivationFunctionType.Exp,
                            scale=float(SCALE))
                        es.append(e)

                    # PV with ones-fold: [65, 512] = [O^T_A | O^T_B ; l]
                    pv = pvps.tile([65, 512], F32, tag="pv")
                    for nh, c0 in ((nA, 0), (nB, 256)):
                        for tt in range(4):
                            nc.tensor.matmul(
                                pv[:, c0:c0 + 256],
                                VA[p][tt][:, nh * 65:(nh + 1) * 65],
                                es[tt][:, c0:c0 + 256],
                                start=(tt == 0), stop=(tt == 3))

                    # softmax denominators -> broadcast reciprocal
                    l2 = lp.tile([1, 512], F32, tag="l2")
                    nc.vector.tensor_copy(l2[0:1, 0:256], pv[64:65, 0:256])
                    nc.vector.tensor_copy(l2[0:1, 256:512], pv[64:65, 256:512])
                    r2 = lp.tile([1, 512], F32R, tag="r2")
                    with nc.allow_low_precision(reason="fp32r matmul input"):
                        nc.vector.reciprocal(r2[:], l2[:])
                    rb = rps.tile([64, 512], F32, tag="rb")
                    nc.tensor.matmul(rb[:, 0:256], on_sb[:], r2[0:1, 0:256],
                                     start=True, stop=True)
                    nc.tensor.matmul(rb[:, 256:512], on_sb[:], r2[0:1, 256:512],
                                     start=True, stop=True)
                    rsb = rp.tile([64, 512], F32, tag="rsb")
                    nc.vector.tensor_copy(rsb[:], rb[:])

                    # normalize + scatter to O^T tiles
                    for h, c0 in ((hA, 0), (hB, 256)):
                        ot = OT[p][h // 2]
                        ob = (h % 2) * 64
                        nc.vector.tensor_tensor(
                            ot[ob:ob + 64, :],
                            pv[0:64, c0:c0 + 256],
                            rsb[0:64, c0:c0 + 256],
                            mybir.AluOpType.mult)

        # ---------- Phase C: output projection ----------
        with ExitStack() as ph:
            wop = ph.enter_context(tc.tile_pool(name="wop", bufs=3))
            yst = ph.enter_context(tc.tile_pool(name="yst", bufs=4))
            psC = ph.enter_context(tc.tile_pool(name="psC", bufs=8, space="PSUM"))

            for nn in range(4):
                acc = [[psC.tile([128, 512], F32, tag="pc", name=f"pc{nn}_{m}") for m in range(2)]
                       for p in range(P)]
                for k in range(16):
                    wt = wop.tile([128, 512], F32R, tag="wo")
                    nc.sync.dma_start(
                        wt[:], wo[k * 128:(k + 1) * 128, nn * 512:(nn + 1) * 512])
                    for p in range(P):
                        for m in range(2):
                            nc.tensor.matmul(
                                acc[p][m][:],
                                OT[p][k][:, m * 128:(m + 1) * 128],
                                wt[:],
                                start=(k == 0), stop=(k == 15))
                for p in range(P):
                    for m in range(2):
                        yt = yst.tile([128, 512], F32, tag="yt")
                        nc.vector.tensor_copy(yt[:], acc[p][m][:])
                        r0 = p * 256 + m * 128
                        nc.sync.dma_start(
                            y[r0:r0 + 128, nn * 512:(nn + 1) * 512], yt[:])

    nc.compile()
    return nc


def _get_nc():
    if "nc" not in _CACHE:
        _CACHE["nc"] = _build()
    return _CACHE["nc"]


def kernel(x, wq, wkv, wo):
    from concourse.bass_utils import run_bass_kernel_spmd

    nc = _get_nc()
    x = np.asarray(x, dtype=np.float32)
    wq_r = _to_f32r(wq)
    wkv_r = _to_f32r(wkv)
    wo_r = _to_f32r(wo)
    ones1 = np.ones((1, HD), np.float32)
    vones = np.ones((128, 8), np.float32)

    in_maps = []
    for c in range(N_CORES):
        xq_cols = []
        xkv_cols = []
        for p in range(P):
            pg = 2 * c + p
            b, v = pg // V, pg % V
            xq_cols.append(np.ascontiguousarray(x[b, v].T))
            xkv_cols.append(np.ascontiguousarray(
                np.concatenate([x[b, (v - 1) % V], x[b, (v + 1) % V]], axis=0).T))
        in_maps.append({
            "xqT": _to_f32r(np.concatenate(xq_cols, axis=1)),
            "xkvT": _to_f32r(np.concatenate(xkv_cols, axis=1)),
            "wq": wq_r, "wkv": wkv_r, "wo": wo_r, "ones1": ones1, "vones": vones,
        })

    res = run_bass_kernel_spmd(nc, in_maps, list(range(N_CORES)),
                               trace=False)
    out = np.empty((B, V, S, D), np.float32)
    for c in range(N_CORES):
        yc = res.results[c]["y"]
        for p in range(P):
            pg = 2 * c + p
            b, v = pg // V, pg % V
            out[b, v] = yc[p * S:(p + 1) * S]
    return out


# revision 9
# speedup vs baseline: 1.8857x; 1.2376x over previous
"""CrossViewAttention Trainium2 kernel.

Shards the B*V=16 (batch, view) attention instances across 8 NeuronCores
(2 per core, data-parallel; weights replicated). The circular neighbor
gather (views v-1, v+1) is resolved on the host when slicing per-core
inputs, so no device collectives are needed.

Per core, for each of its 2 pairs:
  Q^T = wq.T @ x^T           (fp32r matmuls, d-contraction)
  K^T = wk.T @ x_kv^T        V = x_kv @ wv  (natural layout, +ones col)
  scores^T[t,s] = K^T.T @ Q^T   per head (GQA: head h uses kv head h//4)
  E = exp(scale*scores^T)    (no max subtraction; scores are O(1))
  [O^T; l] = V_aug.T @ E     (ones column folds the softmax denominator)
  O^T *= broadcast(1/l)      (K=1 ones matmul broadcasts 1/l over hd)
  y = O @ wo
"""
import numpy as np

B, V, S, D = 2, 8, 256, 2048
NH, NKV, KVR = 32, 8, 2
HD = D // NH  # 64
G = NH // NKV  # 4
N_CORES = 8
P = 2  # pairs per core
SCALE = 1.0 / np.sqrt(HD)

_CACHE = {}


def _to_f32r(a: np.ndarray) -> np.ndarray:
    """Round fp32 to the fp32r format (e8m11, RNE): low 12 bits zeroed."""
    u = np.ascontiguousarray(a, dtype=np.float32).view(np.uint32)
    u = (u + 0x7FF + ((u >> 12) & 1)) & 0xFFFFF000
    return u.view(np.float32)


def _build():
    import concourse.bass as bass
    import concourse.tile as tile
    import concourse.mybir as mybir
    from concourse import bacc
    from contextlib import ExitStack

    F32 = mybir.dt.float32
    F32R = mybir.dt.float32r

    nc = bacc.Bacc("TRN2", target_bir_lowering=False, debug=False,
                   num_devices=N_CORES)
    xqT = nc.dram_tensor("xqT", [D, P * S], F32R, kind="ExternalInput").ap()
    xkvT = nc.dram_tensor("xkvT", [D, P * 512], F32R, kind="ExternalInput").ap()
    wq = nc.dram_tensor("wq", [D, D], F32R, kind="ExternalInput").ap()
    wkv = nc.dram_tensor("wkv", [D, 1024], F32R, kind="ExternalInput").ap()
    wo = nc.dram_tensor("wo", [D, D], F32R, kind="ExternalInput").ap()
    ones1 = nc.dram_tensor("ones1", [1, HD], F32R, kind="ExternalInput").ap()
    vones = nc.dram_tensor("vones", [128, 8], F32R, kind="ExternalInput").ap()
    y = nc.dram_tensor("y", [P * S, D], F32, kind="ExternalOutput").ap()

    with tile.TileContext(nc) as tc, ExitStack() as top:
        misc = top.enter_context(tc.tile_pool(name="misc", bufs=2))
        ktp = top.enter_context(tc.tile_pool(name="ktp", bufs=1))
        vp = top.enter_context(tc.tile_pool(name="vp", bufs=1))

        on_sb = misc.tile([1, HD], F32R, tag="ones")
        nc.gpsimd.dma_start(on_sb[:], ones1[:])
        vo_sb = misc.tile([128, 8], F32R, tag="vones")
        nc.gpsimd.dma_start(vo_sb[:], vones[:])

        KT = [ktp.tile([64, 2048], F32R, tag=f"kt{i}", name=f"kt{i}") for i in range(4)]
        VA = [[vp.tile([128, 8 * 65], F32R, tag=f"va{p}_{t}", name=f"va{p}_{t}") for t in range(4)]
              for p in range(P)]

        # ---------- Phase A1/A2: K^T, V (uses xkvT; wv resident) ----------
        with ExitStack() as ph:
            xkp = ph.enter_context(tc.tile_pool(name="xkp", bufs=1))
            wvp = ph.enter_context(tc.tile_pool(name="wvp", bufs=6))
            wst = ph.enter_context(tc.tile_pool(name="wst", bufs=6))
            psA = ph.enter_context(tc.tile_pool(name="psA", bufs=8, space="PSUM"))

            xkv = []
            for k in range(16):
                t = xkp.tile([128, 1024], F32R, tag=f"xkv{k}", name=f"xkv{k}")
                nc.sync.dma_start(t[:], xkvT[k * 128:(k + 1) * 128, :])
                xkv.append(t)

            # A1: K^T[f, t]; k outer, batched wk loads, 8 accumulators
            kps = [psA.tile([128, 512], F32, tag="pa", name=f"kps{i}")
                   for i in range(8)]
            for k in range(16):
                wt = wst.tile([128, 512], F32R, tag="wk")
                nc.sync.dma_start(wt[:], wkv[k * 128:(k + 1) * 128, 0:512])
                for fk in range(4):
                    nc.tensor.matmul(kps[fk * 2][:],
                                     wt[:, fk * 128:(fk + 1) * 128],
                                     xkv[k][:, 0:512],
                                     start=(k == 0), stop=(k == 15))
                    nc.tensor.matmul(kps[fk * 2 + 1][:],
                                     wt[:, fk * 128:(fk + 1) * 128],
                                     xkv[k][:, 512:1024],
                                     start=(k == 0), stop=(k == 15))
            for fk in range(4):
                ps0, ps1 = kps[fk * 2], kps[fk * 2 + 1]
                nc.vector.tensor_copy(KT[fk][0:64, 0:512], ps0[0:64, :])
                nc.vector.tensor_copy(KT[fk][0:64, 1024:1536], ps0[64:128, :])
                nc.vector.tensor_copy(KT[fk][0:64, 512:1024], ps1[0:64, :])
                nc.vector.tensor_copy(KT[fk][0:64, 1536:2048], ps1[64:128, :])

            # A2: V natural [t, f] + ones; k-outer per pair, wv streamed 2x
            for p in range(P):
                vps = [psA.tile([128, 512], F32, tag="pa", name=f"pvv{p}_{i}")
                       for i in range(4)]
                for k in range(16):
                    wvt = wvp.tile([128, 512], F32R, tag="wv")
                    nc.sync.dma_start(wvt[:], wkv[k * 128:(k + 1) * 128, 512:1024])
                    for tt in range(4):
                        nc.tensor.matmul(
                            vps[tt][:],
                            xkv[k][:, p * 512 + tt * 128:p * 512 + (tt + 1) * 128],
                            wvt[:],
                            start=(k == 0), stop=(k == 15))
                for tt in range(4):
                    ps = vps[tt]
                    va = VA[p][tt]
                    dst = va[:].rearrange("q (h c) -> q h c", c=65)[:, :, 0:64]
                    src = ps[:].rearrange("q (h c) -> q h c", c=64)
                    nc.vector.tensor_copy(dst, src)
                    od = va[:].rearrange("q (h c) -> q h c", c=65)[:, :, 64:65]
                    nc.vector.tensor_copy(od, vo_sb[:].unsqueeze(2))

        # ---------- Phase A3: Q^T (uses xqT) ----------
        qtp = top.enter_context(tc.tile_pool(name="qtp", bufs=1))
        QT = [qtp.tile([64, 1024], F32R, tag=f"qt{j}", name=f"qt{j}")
              for j in range(16)]
        otp = top.enter_context(tc.tile_pool(name="otp", bufs=1))
        OT = [[otp.tile([128, 256], F32R, tag=f"ot{p}_{i}", name=f"ot{p}_{i}") for i in range(16)]
              for p in range(P)]

        with ExitStack() as ph:
            xqp = ph.enter_context(tc.tile_pool(name="xqp", bufs=1))
            wst = ph.enter_context(tc.tile_pool(name="wst2", bufs=6))
            psA = ph.enter_context(tc.tile_pool(name="psA2", bufs=8, space="PSUM"))

            xq = []
            for k in range(16):
                t = xqp.tile([128, 512], F32R, tag=f"xq{k}", name=f"xq{k}")
                nc.sync.dma_start(t[:], xqT[k * 128:(k + 1) * 128, :])
                xq.append(t)

            for fg in range(4):
                qps = [psA.tile([128, 512], F32, tag="pa", name=f"qps{fg}_{i}")
                       for i in range(4)]
                for k in range(16):
                    wt = wst.tile([128, 512], F32R, tag="wq")
                    nc.sync.dma_start(
                        wt[:], wq[k * 128:(k + 1) * 128, fg * 512:(fg + 1) * 512])
                    for fi in range(4):
                        nc.tensor.matmul(qps[fi][:],
                                         wt[:, fi * 128:(fi + 1) * 128],
                                         xq[k][:],
                                         start=(k == 0), stop=(k == 15))
                for fi in range(4):
                    fq = fg * 4 + fi
                    ps = qps[fi]
                    for half, h in ((0, 2 * fq), (1, 2 * fq + 1)):
                        n, g = h // 4, h % 4
                        j = (n // 2) * 4 + g
                        blk = (n % 2) * 512
                        nc.vector.tensor_copy(
                            QT[j][0:64, blk:blk + 512],
                            ps[half * 64:(half + 1) * 64, :])

        # ---------- Phase B: attention ----------
        with ExitStack() as ph:
            ep = ph.enter_context(tc.tile_pool(name="ep", bufs=8))
            lp = ph.enter_context(tc.tile_pool(name="lp", bufs=4))
            rp = ph.enter_context(tc.tile_pool(name="rp", bufs=4))
            qkps = ph.enter_context(tc.tile_pool(name="qkps", bufs=5, space="PSUM"))
            pvps = ph.enter_context(tc.tile_pool(name="pvps", bufs=2, space="PSUM"))
            rps = ph.enter_context(tc.tile_pool(name="rps", bufs=1, space="PSUM"))

            for j in range(16):
                a = j // 4
                g = j % 4
                jk = j // 4
                hA = 8 * a + g
                hB = 8 * a + 4 + g
                nA = 2 * a
                nB = 2 * a + 1
                for p in range(P):
                    # QK^T: per t-tile, heads A (rows 0:64) and B (64:128)
                    es = []
                    for tt in range(4):
                        qk = qkps.tile([128, 512], F32, tag="qk")
                        cA = p * 512 + tt * 128
                        nc.tensor.matmul(
                            qk[:, 0:256],
                            KT[jk][0:64, cA:cA + 128],
                            QT[j][0:64, p * 256:(p + 1) * 256],
                            start=True, stop=True)
                        nc.tensor.matmul(
                            qk[:, 256:512],
                            KT[jk][0:64, 1024 + cA:1024 + cA + 128],
                            QT[j][0:64, 512 + p * 256:512 + (p + 1) * 256],
                            start=True, stop=True)
                        e = ep.tile([128, 512], F32R, tag="e")
                        nc.scalar.activation(
                            e[:], qk[:], mybir.ActivationFunctionType.Exp,
                            scale=float(SCALE))
                        es.append(e)

                    # PV with ones-fold: [65, 512] = [O^T_A | O^T_B ; l]
                    pv = pvps.tile([65, 512], F32, tag="pv")
                    for nh, c0 in ((nA, 0), (nB, 256)):
                        for tt in range(4):
                            nc.tensor.matmul(
                                pv[:, c0:c0 + 256],
                                VA[p][tt][:, nh * 65:(nh + 1) * 65],
                                es[tt][:, c0:c0 + 256],
                                start=(tt == 0), stop=(tt == 3))

                    # softmax denominators -> broadcast reciprocal
                    l2 = lp.tile([1, 512], F32, tag="l2")
                    nc.vector.tensor_copy(l2[0:1, 0:256], pv[64:65, 0:256])
                    nc.vector.tensor_copy(l2[0:1, 256:512], pv[64:65, 256:512])
                    r2 = lp.tile([1, 512], F32R, tag="r2")
                    with nc.allow_low_precision(reason="fp32r matmul input"):
                        nc.vector.reciprocal(r2[:], l2[:])
                    rb = rps.tile([64, 512], F32, tag="rb")
                    nc.tensor.matmul(rb[:, 0:256], on_sb[:], r2[0:1, 0:256],
                                     start=True, stop=True)
                    nc.tensor.matmul(rb[:, 256:512], on_sb[:], r2[0:1, 256:512],
                                     start=True, stop=True)
                    rsb = rp.tile([64, 512], F32, tag="rsb")
                    nc.vector.tensor_copy(rsb[:], rb[:])

                    # normalize + scatter to O^T tiles
                    for h, c0 in ((hA, 0), (hB, 256)):
                        ot = OT[p][h // 2]
                        ob = (h % 2) * 64
                        nc.vector.tensor_tensor(
                            ot[ob:ob + 64, :],
                            pv[0:64, c0:c0 + 256],
                            rsb[0:64, c0:c0 + 256],
                            mybir.AluOpType.mult)

        # ---------- Phase C: output projection ----------
        with ExitStack() as ph:
            wop = ph.enter_context(tc.tile_pool(name="wop", bufs=6))
            yst = ph.enter_context(tc.tile_pool(name="yst", bufs=4))
            psC = ph.enter_context(tc.tile_pool(name="psC", bufs=8, space="PSUM"))

            for nn in range(4):
                acc = [[psC.tile([128, 512], F32, tag="pc", name=f"pc{nn}_{m}") for m in range(2)]
                       for p in range(P)]
                for k in range(16):
                    wt = wop.tile([128, 512], F32R, tag="wo")
                    nc.sync.dma_start(
                        wt[:], wo[k * 128:(k + 1) * 128, nn * 512:(nn + 1) * 512])
                    for p in range(P):
                        for m in range(2):
                            nc.tensor.matmul(
                                acc[p][m][:],
                                OT[p][k][:, m * 128:(m + 1) * 128],
                                wt[:],
                                start=(k == 0), stop=(k == 15))
                for p in range(P):
                    for m in range(2):
                        yt = yst.tile([128, 512], F32, tag="yt")
                        nc.vector.tensor_copy(yt[:], acc[p][m][:])
                        r0 = p * 256 + m * 128
                        nc.sync.dma_start(
                            y[r0:r0 + 128, nn * 512:(nn + 1) * 512], yt[:])

    nc.compile()
    return nc


def _get_nc():
    if "nc" not in _CACHE:
        _CACHE["nc"] = _build()
    return _CACHE["nc"]


def kernel(x, wq, wkv, wo):
    from concourse.bass_utils import run_bass_kernel_spmd

    nc = _get_nc()
    x = np.asarray(x, dtype=np.float32)
    wq_r = _to_f32r(wq)
    wkv_r = _to_f32r(wkv)
    wo_r = _to_f32r(wo)
    ones1 = np.ones((1, HD), np.float32)
    vones = np.ones((128, 8), np.float32)

    in_maps = []
    for c in range(N_CORES):
        xq_cols = []
        xkv_cols = []
        for p in range(P):
            pg = 2 * c + p
            b, v = pg // V, pg % V
            xq_cols.append(np.ascontiguousarray(x[b, v].T))
            xkv_cols.append(np.ascontiguousarray(
                np.concatenate([x[b, (v - 1) % V], x[b, (v + 1) % V]], axis=0).T))
        in_maps.append({
            "xqT": _to_f32r(np.concatenate(xq_cols, axis=1)),
            "xkvT": _to_f32r(np.concatenate(xkv_cols, axis=1)),
            "wq": wq_r, "wkv": wkv_r, "wo": wo_r, "ones1": ones1, "vones": vones,
        })

    res = run_bass_kernel_spmd(nc, in_maps, list(range(N_CORES)),
                               trace=False)
    out = np.empty((B, V, S, D), np.float32)
    for c in range(N_CORES):
        yc = res.results[c]["y"]
        for p in range(P):
            pg = 2 * c + p
            b, v = pg // V, pg % V
            out[b, v] = yc[p * S:(p + 1) * S]
    return out


# revision 10
# speedup vs baseline: 1.8888x; 1.0017x over previous
"""CrossViewAttention Trainium2 kernel.

Shards the B*V=16 (batch, view) attention instances across 8 NeuronCores
(2 per core, data-parallel; weights replicated). The circular neighbor
gather (views v-1, v+1) is resolved on the host when slicing per-core
inputs, so no device collectives are needed.

Per core, for each of its 2 pairs:
  Q^T = wq.T @ x^T           (fp32r matmuls, d-contraction)
  K^T = wk.T @ x_kv^T        V = x_kv @ wv  (natural layout, +ones col)
  scores^T[t,s] = K^T.T @ Q^T   per head (GQA: head h uses kv head h//4)
  E = exp(scale*scores^T)    (no max subtraction; scores are O(1))
  [O^T; l] = V_aug.T @ E     (ones column folds the softmax denominator)
  O^T *= broadcast(1/l)      (K=1 ones matmul broadcasts 1/l over hd)
  y = O @ wo
"""
import numpy as np

B, V, S, D = 2, 8, 256, 2048
NH, NKV, KVR = 32, 8, 2
HD = D // NH  # 64
G = NH // NKV  # 4
N_CORES = 8
P = 2  # pairs per core
SCALE = 1.0 / np.sqrt(HD)

_CACHE = {}


def _to_f32r(a: np.ndarray) -> np.ndarray:
    """Round fp32 to the fp32r format (e8m11, RNE): low 12 bits zeroed."""
    u = np.ascontiguousarray(a, dtype=np.float32).view(np.uint32)
    u = (u + 0x7FF + ((u >> 12) & 1)) & 0xFFFFF000
    return u.view(np.float32)


def _build():
    import concourse.bass as bass
    import concourse.tile as tile
    import concourse.mybir as mybir
    from concourse import bacc
    from contextlib import ExitStack

    F32 = mybir.dt.float32
    F32R = mybir.dt.float32r

    nc = bacc.Bacc("TRN2", target_bir_lowering=False, debug=False,
                   num_devices=N_CORES)
    xqT = nc.dram_tensor("xqT", [D, P * S], F32R, kind="ExternalInput").ap()
    xkvT = nc.dram_tensor("xkvT", [D, P * 512], F32R, kind="ExternalInput").ap()
    wq = nc.dram_tensor("wq", [D, D], F32R, kind="ExternalInput").ap()
    wkv = nc.dram_tensor("wkv", [D, 1024], F32R, kind="ExternalInput").ap()
    wo = nc.dram_tensor("wo", [D, D], F32R, kind="ExternalInput").ap()
    ones1 = nc.dram_tensor("ones1", [1, HD], F32R, kind="ExternalInput").ap()
    vones = nc.dram_tensor("vones", [128, 8], F32R, kind="ExternalInput").ap()
    y = nc.dram_tensor("y", [P * S, D], F32, kind="ExternalOutput").ap()

    with tile.TileContext(nc) as tc, ExitStack() as top:
        misc = top.enter_context(tc.tile_pool(name="misc", bufs=2))
        ktp = top.enter_context(tc.tile_pool(name="ktp", bufs=1))
        vp = top.enter_context(tc.tile_pool(name="vp", bufs=1))

        on_sb = misc.tile([1, HD], F32R, tag="ones")
        nc.gpsimd.dma_start(on_sb[:], ones1[:])
        vo_sb = misc.tile([128, 8], F32R, tag="vones")
        nc.gpsimd.dma_start(vo_sb[:], vones[:])

        KT = [ktp.tile([64, 2048], F32R, tag=f"kt{i}", name=f"kt{i}") for i in range(4)]
        VA = [[vp.tile([128, 8 * 65], F32R, tag=f"va{p}_{t}", name=f"va{p}_{t}") for t in range(4)]
              for p in range(P)]

        # ---------- Phase A1/A2: K^T, V (uses xkvT; wv resident) ----------
        with ExitStack() as ph:
            xkp = ph.enter_context(tc.tile_pool(name="xkp", bufs=1))
            wvp = ph.enter_context(tc.tile_pool(name="wvp", bufs=6))
            wst = ph.enter_context(tc.tile_pool(name="wst", bufs=6))
            psA = ph.enter_context(tc.tile_pool(name="psA", bufs=8, space="PSUM"))

            xkv = []
            for k in range(16):
                t = xkp.tile([128, 1024], F32R, tag=f"xkv{k}", name=f"xkv{k}")
                nc.sync.dma_start(t[:], xkvT[k * 128:(k + 1) * 128, :])
                xkv.append(t)

            # A1: K^T[f, t]; k outer, batched wk loads, 8 accumulators
            kps = [psA.tile([128, 512], F32, tag="pa", name=f"kps{i}")
                   for i in range(8)]
            for k in range(16):
                wt = wst.tile([128, 512], F32R, tag="wk")
                nc.sync.dma_start(wt[:], wkv[k * 128:(k + 1) * 128, 0:512])
                for fk in range(4):
                    nc.tensor.matmul(kps[fk * 2][:],
                                     wt[:, fk * 128:(fk + 1) * 128],
                                     xkv[k][:, 0:512],
                                     start=(k == 0), stop=(k == 15))
                    nc.tensor.matmul(kps[fk * 2 + 1][:],
                                     wt[:, fk * 128:(fk + 1) * 128],
                                     xkv[k][:, 512:1024],
                                     start=(k == 0), stop=(k == 15))
            for fk in range(4):
                ps0, ps1 = kps[fk * 2], kps[fk * 2 + 1]
                nc.vector.tensor_copy(KT[fk][0:64, 0:512], ps0[0:64, :])
                nc.vector.tensor_copy(KT[fk][0:64, 1024:1536], ps0[64:128, :])
                nc.vector.tensor_copy(KT[fk][0:64, 512:1024], ps1[0:64, :])
                nc.vector.tensor_copy(KT[fk][0:64, 1536:2048], ps1[64:128, :])

            # A2: V natural [t, f] + ones; k-outer per pair, wv streamed 2x
            for p in range(P):
                vps = [psA.tile([128, 512], F32, tag="pa", name=f"pvv{p}_{i}")
                       for i in range(4)]
                for k in range(16):
                    wvt = wvp.tile([128, 512], F32R, tag="wv")
                    nc.sync.dma_start(wvt[:], wkv[k * 128:(k + 1) * 128, 512:1024])
                    for tt in range(4):
                        nc.tensor.matmul(
                            vps[tt][:],
                            xkv[k][:, p * 512 + tt * 128:p * 512 + (tt + 1) * 128],
                            wvt[:],
                            start=(k == 0), stop=(k == 15))
                for tt in range(4):
                    ps = vps[tt]
                    va = VA[p][tt]
                    dst = va[:].rearrange("q (h c) -> q h c", c=65)[:, :, 0:64]
                    src = ps[:].rearrange("q (h c) -> q h c", c=64)
                    nc.vector.tensor_copy(dst, src)
                    od = va[:].rearrange("q (h c) -> q h c", c=65)[:, :, 64:65]
                    nc.vector.tensor_copy(od, vo_sb[:].unsqueeze(2))

        # ---------- Phase A3: Q^T (uses xqT) ----------
        qtp = top.enter_context(tc.tile_pool(name="qtp", bufs=1))
        QT = [qtp.tile([64, 1024], F32R, tag=f"qt{j}", name=f"qt{j}")
              for j in range(16)]
        otp = top.enter_context(tc.tile_pool(name="otp", bufs=1))
        OT = [[otp.tile([128, 256], F32R, tag=f"ot{p}_{i}", name=f"ot{p}_{i}") for i in range(16)]
              for p in range(P)]

        with ExitStack() as ph:
            xqp = ph.enter_context(tc.tile_pool(name="xqp", bufs=1))
            wst = ph.enter_context(tc.tile_pool(name="wst2", bufs=6))
            psA = ph.enter_context(tc.tile_pool(name="psA2", bufs=8, space="PSUM"))

            xq = []
            for k in range(16):
                t = xqp.tile([128, 512], F32R, tag=f"xq{k}", name=f"xq{k}")
                nc.sync.dma_start(t[:], xqT[k * 128:(k + 1) * 128, :])
                xq.append(t)

            for fg in range(4):
                qps = [psA.tile([128, 512], F32, tag="pa", name=f"qps{fg}_{i}")
                       for i in range(4)]
                for k in range(16):
                    wt = wst.tile([128, 512], F32R, tag="wq")
                    nc.sync.dma_start(
                        wt[:], wq[k * 128:(k + 1) * 128, fg * 512:(fg + 1) * 512])
                    for fi in range(4):
                        nc.tensor.matmul(qps[fi][:],
                                         wt[:, fi * 128:(fi + 1) * 128],
                                         xq[k][:],
                                         start=(k == 0), stop=(k == 15))
                for fi in range(4):
                    fq = fg * 4 + fi
                    ps = qps[fi]
                    nc.vector.tensor_copy(QT[fq][0:64, 0:512], ps[0:64, :])
                    nc.vector.tensor_copy(QT[fq][0:64, 512:1024], ps[64:128, :])

        # ---------- Phase B: attention ----------
        with ExitStack() as ph:
            ep = ph.enter_context(tc.tile_pool(name="ep", bufs=8))
            lp = ph.enter_context(tc.tile_pool(name="lp", bufs=4))
            rp = ph.enter_context(tc.tile_pool(name="rp", bufs=4))
            qkps = ph.enter_context(tc.tile_pool(name="qkps", bufs=5, space="PSUM"))
            pvps = ph.enter_context(tc.tile_pool(name="pvps", bufs=2, space="PSUM"))
            rps = ph.enter_context(tc.tile_pool(name="rps", bufs=1, space="PSUM"))

            for j in range(16):
                n = j // 2
                u = j % 2
                jk = n // 2
                nhalf = n % 2
                hA = 4 * n + 2 * u
                hB = hA + 1
                for p in range(P):
                    # QK^T: per t-tile, heads A (rows 0:64) and B (64:128)
                    es = []
                    for tt in range(4):
                        qk = qkps.tile([128, 512], F32, tag="qk")
                        cA = nhalf * 1024 + p * 512 + tt * 128
                        nc.tensor.matmul(
                            qk[:, 0:256],
                            KT[jk][0:64, cA:cA + 128],
                            QT[j][0:64, p * 256:(p + 1) * 256],
                            start=True, stop=True)
                        nc.tensor.matmul(
                            qk[:, 256:512],
                            KT[jk][0:64, cA:cA + 128],
                            QT[j][0:64, 512 + p * 256:512 + (p + 1) * 256],
                            start=True, stop=True)
                        e = ep.tile([128, 512], F32R, tag="e")
                        nc.scalar.activation(
                            e[:], qk[:], mybir.ActivationFunctionType.Exp,
                            scale=float(SCALE))
                        es.append(e)

                    # PV with ones-fold: [65, 512] = [O^T_A | O^T_B ; l]
                    pv = pvps.tile([65, 512], F32, tag="pv")
                    for tt in range(4):
                        nc.tensor.matmul(
                            pv[:, 0:512],
                            VA[p][tt][:, n * 65:(n + 1) * 65],
                            es[tt][:, 0:512],
                            start=(tt == 0), stop=(tt == 3))

                    # softmax denominators -> broadcast reciprocal
                    l2 = lp.tile([1, 512], F32, tag="l2")
                    nc.vector.tensor_copy(l2[0:1, 0:256], pv[64:65, 0:256])
                    nc.vector.tensor_copy(l2[0:1, 256:512], pv[64:65, 256:512])
                    r2 = lp.tile([1, 512], F32R, tag="r2")
                    with nc.allow_low_precision(reason="fp32r matmul input"):
                        nc.vector.reciprocal(r2[:], l2[:])
                    rb = rps.tile([64, 512], F32, tag="rb")
                    nc.tensor.matmul(rb[:, 0:256], on_sb[:], r2[0:1, 0:256],
                                     start=True, stop=True)
                    nc.tensor.matmul(rb[:, 256:512], on_sb[:], r2[0:1, 256:512],
                                     start=True, stop=True)
                    rsb = rp.tile([64, 512], F32, tag="rsb")
                    nc.vector.tensor_copy(rsb[:], rb[:])

                    # normalize + scatter to O^T tiles
                    for h, c0 in ((hA, 0), (hB, 256)):
                        ot = OT[p][h // 2]
                        ob = (h % 2) * 64
                        nc.vector.tensor_tensor(
                            ot[ob:ob + 64, :],
                            pv[0:64, c0:c0 + 256],
                            rsb[0:64, c0:c0 + 256],
                            mybir.AluOpType.mult)

        # ---------- Phase C: output projection ----------
        with ExitStack() as ph:
            wop = ph.enter_context(tc.tile_pool(name="wop", bufs=6))
            yst = ph.enter_context(tc.tile_pool(name="yst", bufs=4))
            psC = ph.enter_context(tc.tile_pool(name="psC", bufs=8, space="PSUM"))

            for nn in range(4):
                acc = [[psC.tile([128, 512], F32, tag="pc", name=f"pc{nn}_{m}") for m in range(2)]
                       for p in range(P)]
                for k in range(16):
                    wt = wop.tile([128, 512], F32R, tag="wo")
                    nc.sync.dma_start(
                        wt[:], wo[k * 128:(k + 1) * 128, nn * 512:(nn + 1) * 512])
                    for p in range(P):
                        for m in range(2):
                            nc.tensor.matmul(
                                acc[p][m][:],
                                OT[p][k][:, m * 128:(m + 1) * 128],
                                wt[:],
                                start=(k == 0), stop=(k == 15))
                for p in range(P):
                    for m in range(2):
                        yt = yst.tile([128, 512], F32, tag="yt")
                        nc.vector.tensor_copy(yt[:], acc[p][m][:])
                        r0 = p * 256 + m * 128
                        nc.sync.dma_start(
                            y[r0:r0 + 128, nn * 512:(nn + 1) * 512], yt[:])

    nc.compile()
    return nc


def _get_nc():
    if "nc" not in _CACHE:
        _CACHE["nc"] = _build()
    return _CACHE["nc"]


def kernel(x, wq, wkv, wo):
    from concourse.bass_utils import run_bass_kernel_spmd

    nc = _get_nc()
    x = np.asarray(x, dtype=np.float32)
    wq_r = _to_f32r(wq)
    wkv_r = _to_f32r(wkv)
    wo_r = _to_f32r(wo)
    ones1 = np.ones((1, HD), np.float32)
    vones = np.ones((128, 8), np.float32)

    in_maps = []
    for c in range(N_CORES):
        xq_cols = []
        xkv_cols = []
        for p in range(P):
            pg = 2 * c + p
            b, v = pg // V, pg % V
            xq_cols.append(np.ascontiguousarray(x[b, v].T))
            xkv_cols.append(np.ascontiguousarray(
                np.concatenate([x[b, (v - 1) % V], x[b, (v + 1) % V]], axis=0).T))
        in_maps.append({
            "xqT": _to_f32r(np.concatenate(xq_cols, axis=1)),
            "xkvT": _to_f32r(np.concatenate(xkv_cols, axis=1)),
            "wq": wq_r, "wkv": wkv_r, "wo": wo_r, "ones1": ones1, "vones": vones,
        })

    res = run_bass_kernel_spmd(nc, in_maps, list(range(N_CORES)),
                               trace=False)
    out = np.empty((B, V, S, D), np.float32)
    for c in range(N_CORES):
        yc = res.results[c]["y"]
        for p in range(P):
            pg = 2 * c + p
            b, v = pg // V, pg % V
            out[b, v] = yc[p * S:(p + 1) * S]
    return out
